# revision 7
# baseline (speedup 1.0000x reference)
"""Trainium2 Bass kernel for a bag-of-words model (EmbeddingBag mean ->
Linear -> BatchNorm(train stats) -> ReLU -> Linear).

Strategy (8 NeuronCores, SPMD):
  - Data-parallel over batch: 1024 examples -> 128 per core, assignment
    balanced by token count. Embedding table replicated per core (bf16).
  - Host compacts the ragged token lists (drops padding), buckets tokens by
    vocab range of 32768 rows so indices fit dma_gather's int16, and builds
    per-token-slot segment ids.
  - Device: dma_gather pulls only the valid embedding rows into SBUF
    (128 token-slots per partition-tile); pooling is done on TensorE as
    pooled[seg, H] += M_tile^T-free @ G_tile with M built on DVE via
    is_equal(iota, seg_id) (an exact 0/1 mask), accumulating in PSUM.
  - MLP: pooled/len -> PE transpose -> h^T = W1^T-natural matmuls (+b1),
    BN batch stats via free-dim reduction + a 4KB AllReduce across cores,
    fused scale/shift/ReLU on ACT, final 5-wide matmul, +b2, DMA out.
"""

import os
import numpy as np
import ml_dtypes

B, L, V, H = 1024, 200, 100000, 512
NCORES = 8
PCORE = B // NCORES  # 128 examples per core
BUCKET = 32768  # dma_gather int16 index range per bucket
NBUCKETS = (V + BUCKET - 1) // BUCKET  # 4
BN_EPS = 1e-5
CHUNK_TILES = 8  # token-tiles per dma_gather call (HW caps ~1024 idx/call)
SEG_PAD = 200.0  # segment id for padding slots (never matches 0..127)
EMB_DT_BF16 = True  # gather/pool in bf16 (halves HBM traffic)

_CACHE = {}


# ----------------------------------------------------------------- host prep
def _assign_cores(lengths):
    """Balanced assignment: 128 examples per core, ~equal total tokens."""
    order = np.argsort(-lengths, kind="stable")
    loads = [0] * NCORES
    counts = [0] * NCORES
    cores = [[] for _ in range(NCORES)]
    for ex in order:
        c = min(
            (c for c in range(NCORES) if counts[c] < PCORE),
            key=lambda c: loads[c],
        )
        cores[c].append(int(ex))
        loads[c] += int(lengths[ex])
        counts[c] += 1
    return cores  # cores[c] = list of 128 original example indices


def _prep(tokens, lengths):
    """Build per-core gather indices / segment ids / call plan."""
    cores = _assign_cores(lengths)

    # per (core, bucket): list of (idx16, seg)
    percb = [[[] for _ in range(NBUCKETS)] for _ in range(NCORES)]
    for c in range(NCORES):
        for slot, ex in enumerate(cores[c]):
            n = int(lengths[ex])
            toks = tokens[ex, :n]
            bs = toks >> 15
            rs = toks & 0x7FFF
            for b, r in zip(bs, rs):
                percb[c][b].append((int(r), slot))

    # shared padded bucket sizes (tiles), so all cores run the same program
    bsz = []
    for b in range(NBUCKETS):
        mx = max(len(percb[c][b]) for c in range(NCORES))
        bsz.append(-(-mx // 128) * 128)  # round up to full 128-slot tiles
    n_slots = sum(bsz)
    n_tiles = n_slots // 128

    # gather call plan: (bucket_base_row, tile_offset, n_tiles_call)
    calls = []
    t0 = 0
    for b in range(NBUCKETS):
        bt = bsz[b] // 128
        done = 0
        while done < bt:
            nt = min(CHUNK_TILES, bt - done)
            calls.append((b * BUCKET, t0 + done, nt))
            done += nt
        t0 += bt

    idx16 = np.zeros((NCORES, 128, n_slots // 16), dtype=np.int16)
    seg = np.full((NCORES, 128, n_tiles), SEG_PAD, dtype=np.float32)
    leninv = np.zeros((NCORES, 128, 1), dtype=np.float32)
    for c in range(NCORES):
        flat_idx = np.zeros(n_slots, dtype=np.int16)
        flat_seg = np.full(n_slots, SEG_PAD, dtype=np.float32)
        off = 0
        for b in range(NBUCKETS):
            lst = percb[c][b]
            if lst:
                arr = np.asarray(lst, dtype=np.int64)
                flat_idx[off : off + len(lst)] = arr[:, 0].astype(np.int16)
                flat_seg[off : off + len(lst)] = arr[:, 1]
            off += bsz[b]
        # wrap: slot k -> [k % 16, k // 16], replicated to 128 partitions
        w = flat_idx.reshape(n_slots // 16, 16).T  # [16, n_slots//16]
        idx16[c] = np.tile(w, (8, 1))
        # seg: slot j*128+p -> [p, j]
        seg[c] = flat_seg.reshape(n_tiles, 128).T.astype(np.float32)
        leninv[c, :, 0] = 1.0 / lengths[np.asarray(cores[c])].astype(np.float32)

    return cores, calls, n_tiles, idx16, seg, leninv


# -------------------------------------------------------------- device build
def _build(n_tiles, calls, emb_dt_np):
    import concourse.bacc as bacc
    import concourse.tile as tile
    import concourse.mybir as mybir

    emb_dt = mybir.dt.bfloat16 if emb_dt_np == ml_dtypes.bfloat16 else mybir.dt.float32
    f32 = mybir.dt.float32

    nc = bacc.Bacc(
        "TRN2",
        target_bir_lowering=False,
        debug=False,
        enable_asserts=False,
        num_devices=NCORES,
        dynamic_dma_scratch_size=32768,
    )

    emb_d = nc.dram_tensor("emb", [V, H], emb_dt, kind="ExternalInput")
    idx_d = nc.dram_tensor(
        "idx16", [128, n_tiles * 8], mybir.dt.int16, kind="ExternalInput"
    )
    seg_d = nc.dram_tensor("seg", [128, n_tiles], f32, kind="ExternalInput")
    leninv_d = nc.dram_tensor("leninv", [128, 1], f32, kind="ExternalInput")
    w1_d = nc.dram_tensor("W1", [H, H], f32, kind="ExternalInput")
    b1_d = nc.dram_tensor("b1c", [128, 4], f32, kind="ExternalInput")
    gamma_d = nc.dram_tensor("gammac", [128, 4], f32, kind="ExternalInput")
    beta_d = nc.dram_tensor("betac", [128, 4], f32, kind="ExternalInput")
    w2_d = nc.dram_tensor("W2", [H, 5], f32, kind="ExternalInput")
    b2_d = nc.dram_tensor("b2rep", [128, 5], f32, kind="ExternalInput")
    iota_d = nc.dram_tensor("iota", [128, 128], emb_dt, kind="ExternalInput")
    ident_d = nc.dram_tensor("ident", [128, 128], f32, kind="ExternalInput")
    out_d = nc.dram_tensor("out", [128, 5], f32, kind="ExternalOutput")

    with tile.TileContext(nc) as tc:
        with (
            tc.tile_pool(name="const", bufs=1) as cpool,
            tc.tile_pool(name="gbuf", bufs=4) as gpool,
            tc.tile_pool(name="mbuf", bufs=4) as mpool,
            tc.tile_pool(name="work", bufs=1) as wpool,
            tc.tile_pool(name="ppool", bufs=1, space="PSUM") as ppool,
            tc.tile_pool(name="tpsum", bufs=2, space="PSUM") as tppool,
            tc.tile_pool(name="opsum", bufs=1, space="PSUM") as opool,
            tc.tile_pool(name="dram", bufs=1, space="DRAM") as dpool,
        ):
            # ---- constant / input loads
            idx_sb = cpool.tile([128, n_tiles * 8], mybir.dt.int16, tag="idx")
            seg_sb = cpool.tile([128, n_tiles], f32, tag="seg")
            leninv_sb = cpool.tile([128, 1], f32, tag="leninv")
            iota_sb = cpool.tile([128, 128], emb_dt, tag="iota")
            ident_sb = cpool.tile([128, 128], f32, tag="ident")
            b1_sb = cpool.tile([128, 4], f32, tag="b1")
            gamma_sb = cpool.tile([128, 4], f32, tag="gamma")
            beta_sb = cpool.tile([128, 4], f32, tag="beta")
            b2_sb = cpool.tile([128, 5], f32, tag="b2")
            w2_sb = cpool.tile([128, 4, 5], f32, tag="w2")
            w1_sb = [
                cpool.tile([128, H], f32, tag=f"w1_{k}", name=f"w1_{k}")
                for k in range(4)
            ]

            nc.sync.dma_start(idx_sb[:], idx_d[:, :])
            nc.sync.dma_start(seg_sb[:], seg_d[:, :])
            nc.sync.dma_start(leninv_sb[:], leninv_d[:, :])
            nc.sync.dma_start(iota_sb[:], iota_d[:, :])
            nc.sync.dma_start(ident_sb[:], ident_d[:, :])
            nc.sync.dma_start(b1_sb[:], b1_d[:, :])
            nc.sync.dma_start(gamma_sb[:], gamma_d[:, :])
            nc.sync.dma_start(beta_sb[:], beta_d[:, :])
            nc.sync.dma_start(b2_sb[:], b2_d[:, :])
            for k in range(4):
                nc.sync.dma_start(w1_sb[k][:], w1_d[k * 128 : (k + 1) * 128, :])
                nc.sync.dma_start(w2_sb[:, k, :], w2_d[k * 128 : (k + 1) * 128, :])

            # ---- gather + pooling matmuls (accumulate pooled[seg, H] in PSUM)
            pooled_ps = ppool.tile([128, H], f32, tag="pooled")
            emb_ap = emb_d.ap()
            tile_idx = 0
            n_calls = len(calls)
            for ci, (base, t0, nt) in enumerate(calls):
                rows = min(BUCKET, V - base)
                g_sb = gpool.tile([128, CHUNK_TILES, H], emb_dt, tag="g")
                nidx = nt * 128
                nc.gpsimd.dma_gather(
                    out_ap=g_sb[:, :nt, :],
                    in_ap=emb_ap[base : base + rows, :],
                    idxs_ap=idx_sb[:, t0 * 8 : (t0 + nt) * 8],
                    num_idxs=nidx,
                    num_idxs_reg=nidx,
                    elem_size=H,
                )
                for j in range(nt):
                    t = t0 + j
                    m_sb = mpool.tile([128, 128], emb_dt, tag="m")
                    nc.vector.tensor_scalar(
                        out=m_sb[:],
                        in0=iota_sb[:],
                        scalar1=seg_sb[:, t : t + 1],
                        scalar2=None,
                        op0=mybir.AluOpType.is_equal,
                    )
                    nc.tensor.matmul(
                        pooled_ps[:],
                        lhsT=m_sb[:],
                        rhs=g_sb[:, j, :],
                        start=(tile_idx == 0),
                        stop=(tile_idx == n_tiles - 1),
                    )
                    tile_idx += 1

            # ---- pooled mean (x 1/len), transpose to [H, B]
            pooled_sb = wpool.tile([128, H], f32, tag="pooled_sb")
            nc.vector.tensor_scalar_mul(pooled_sb[:], pooled_ps[:], leninv_sb[:, :1])

            pooledT_sb = wpool.tile([128, 4, 128], f32, tag="pooledT")
            for c in range(4):
                tp = tppool.tile([128, 128], f32, tag="tp")
                nc.tensor.transpose(
                    tp[:], pooled_sb[:, c * 128 : (c + 1) * 128], ident_sb[:]
                )
                nc.vector.tensor_copy(pooledT_sb[:, c, :], tp[:])

            # ---- h^T = W1^T @ pooled^T (+b1), stats
            hT_sb = wpool.tile([128, 4, 128], f32, tag="hT")
            stats_sb = wpool.tile([128, 8], f32, tag="stats")
            sq_sb = wpool.tile([128, 128], f32, tag="sq")
            for m in range(4):
                hps = tppool.tile([128, 128], f32, tag="hps")
                for k in range(4):
                    nc.tensor.matmul(
                        hps[:],
                        lhsT=w1_sb[k][:, m * 128 : (m + 1) * 128],
                        rhs=pooledT_sb[:, k, :],
                        start=(k == 0),
                        stop=(k == 3),
                    )
                nc.scalar.activation(
                    out=hT_sb[:, m, :],
                    in_=hps[:],
                    func=mybir.ActivationFunctionType.Identity,
                    bias=b1_sb[:, m : m + 1],
                    scale=1.0,
                )
                nc.vector.reduce_sum(
                    stats_sb[:, m : m + 1],
                    hT_sb[:, m, :],
                    axis=mybir.AxisListType.X,
                )
                nc.scalar.activation(
                    out=sq_sb[:],
                    in_=hT_sb[:, m, :],
                    func=mybir.ActivationFunctionType.Square,
                    accum_out=stats_sb[:, 4 + m : 5 + m],
                )

            # ---- AllReduce the BN partial sums (4KB)
            cc_in = dpool.tile([128, 8], f32, tag="cc_in")
            cc_out = dpool.tile([128, 8], f32, tag="cc_out")
            nc.sync.dma_start(cc_in[:], stats_sb[:])
            nc.gpsimd.collective_compute(
                "AllReduce",
                mybir.AluOpType.add,
                replica_groups=[list(range(NCORES))],
                ins=[cc_in.opt()],
                outs=[cc_out.opt()],
            )
            gstats_sb = wpool.tile([128, 8], f32, tag="gstats")
            nc.sync.dma_start(gstats_sb[:], cc_out[:])

            # ---- BN constants: scale = gamma/sqrt(var+eps), shift = beta - mu*scale
            mu = wpool.tile([128, 4], f32, tag="mu")
            var = wpool.tile([128, 4], f32, tag="var")
            std = wpool.tile([128, 4], f32, tag="std")
            inv = wpool.tile([128, 4], f32, tag="inv")
            scale = wpool.tile([128, 4], f32, tag="scale")
            shift = wpool.tile([128, 4], f32, tag="shift")
            nc.vector.tensor_scalar_mul(mu[:], gstats_sb[:, 0:4], 1.0 / B)
            nc.vector.tensor_scalar_mul(var[:], gstats_sb[:, 4:8], 1.0 / B)
            nc.vector.tensor_tensor(
                out=std[:], in0=mu[:], in1=mu[:], op=mybir.AluOpType.mult
            )
            nc.vector.tensor_tensor(
                out=var[:], in0=var[:], in1=std[:], op=mybir.AluOpType.subtract
            )
            eps_sb = wpool.tile([128, 1], f32, tag="eps")
            nc.vector.memset(eps_sb[:], BN_EPS)
            nc.scalar.activation(
                out=std[:],
                in_=var[:],
                func=mybir.ActivationFunctionType.Sqrt,
                bias=eps_sb[:, :1],
                scale=1.0,
            )
            nc.vector.reciprocal(inv[:], std[:])
            nc.vector.tensor_tensor(
                out=scale[:], in0=inv[:], in1=gamma_sb[:], op=mybir.AluOpType.mult
            )
            nc.vector.tensor_tensor(
                out=shift[:], in0=mu[:], in1=scale[:], op=mybir.AluOpType.mult
            )
            nc.vector.tensor_tensor(
                out=shift[:], in0=beta_sb[:], in1=shift[:], op=mybir.AluOpType.subtract
            )

            # ---- ReLU(h*scale + shift), final matmul, +b2, store
            hn_sb = wpool.tile([128, 4, 128], f32, tag="hn")
            for m in range(4):
                nc.scalar.activation(
                    out=hn_sb[:, m, :],
                    in_=hT_sb[:, m, :],
                    func=mybir.ActivationFunctionType.Relu,
                    bias=shift[:, m : m + 1],
                    scale=scale[:, m : m + 1],
                )
            out_ps = opool.tile([128, 5], f32, tag="out_ps")
            for m in range(4):
                nc.tensor.matmul(
                    out_ps[:],
                    lhsT=hn_sb[:, m, :],
                    rhs=w2_sb[:, m, :],
                    start=(m == 0),
                    stop=(m == 3),
                )
            out_sb = wpool.tile([128, 5], f32, tag="out_sb")
            nc.vector.tensor_tensor(
                out=out_sb[:], in0=out_ps[:], in1=b2_sb[:], op=mybir.AluOpType.add
            )
            nc.sync.dma_start(out_d[:, :], out_sb[:])

    nc.compile()
    return nc


# ------------------------------------------------------------------- runner
def _prepare(inputs):
    tokens = np.asarray(inputs["tokens"], dtype=np.int32)
    lengths = np.asarray(inputs["lengths"], dtype=np.int32)
    emb = np.asarray(inputs["emb"], dtype=np.float32)
    W1 = np.ascontiguousarray(np.asarray(inputs["W1"], dtype=np.float32))
    b1 = np.asarray(inputs["b1"], dtype=np.float32)
    gamma = np.asarray(inputs["gamma"], dtype=np.float32)
    beta = np.asarray(inputs["beta"], dtype=np.float32)
    W2 = np.ascontiguousarray(np.asarray(inputs["W2"], dtype=np.float32))
    b2 = np.asarray(inputs["b2"], dtype=np.float32)

    cores, calls, n_tiles, idx16, seg, leninv = _prep(tokens, lengths)

    emb_dt_np = ml_dtypes.bfloat16 if EMB_DT_BF16 else np.float32
    key = (n_tiles, tuple(calls), EMB_DT_BF16)
    if key not in _CACHE:
        _CACHE[key] = _build(n_tiles, calls, emb_dt_np)
    nc = _CACHE[key]

    emb_c = np.ascontiguousarray(emb.astype(emb_dt_np))
    iota = np.tile(np.arange(128, dtype=np.float32), (128, 1)).astype(emb_dt_np)
    ident = np.eye(128, dtype=np.float32)
    b1c = np.ascontiguousarray(b1.reshape(4, 128).T)
    gammac = np.ascontiguousarray(gamma.reshape(4, 128).T)
    betac = np.ascontiguousarray(beta.reshape(4, 128).T)
    b2rep = np.tile(b2.reshape(1, 5), (128, 1))

    in_maps = []
    for c in range(NCORES):
        in_maps.append(
            {
                "emb": emb_c,
                "idx16": np.ascontiguousarray(idx16[c]),
                "seg": np.ascontiguousarray(seg[c]),
                "leninv": np.ascontiguousarray(leninv[c]),
                "W1": W1,
                "b1c": b1c,
                "gammac": gammac,
                "betac": betac,
                "W2": W2,
                "b2rep": b2rep,
                "iota": iota,
                "ident": ident,
            }
        )
    return nc, in_maps, cores


def _run(inputs, trace=False):
    nc, in_maps, cores = _prepare(inputs)

    from concourse.bass_utils import run_bass_kernel_spmd

    res = run_bass_kernel_spmd(
        nc, in_maps, core_ids=list(range(NCORES)), trace=trace
    )

    out = np.zeros((B, 5), dtype=np.float32)
    for c in range(NCORES):
        out[np.asarray(cores[c])] = res.results[c]["out"]
    return out, res


def kernel(**inputs) -> np.ndarray:
    out, _ = _run(inputs, trace=False)
    return out


# revision 8
# speedup vs baseline: 1.0227x; 1.0227x over previous
"""Trainium2 Bass kernel for a bag-of-words model (EmbeddingBag mean ->
Linear -> BatchNorm(train stats) -> ReLU -> Linear).

Strategy (8 NeuronCores, SPMD):
  - Data-parallel over batch: 1024 examples -> 128 per core, assignment
    balanced by token count. Embedding table replicated per core (bf16).
  - Host compacts the ragged token lists (drops padding), buckets tokens by
    vocab range of 32768 rows so indices fit dma_gather's int16, and builds
    per-token-slot segment ids.
  - Device: dma_gather pulls only the valid embedding rows into SBUF
    (128 token-slots per partition-tile); pooling is done on TensorE as
    pooled[seg, H] += M_tile^T-free @ G_tile with M built on DVE via
    is_equal(iota, seg_id) (an exact 0/1 mask), accumulating in PSUM.
  - MLP: pooled/len -> PE transpose -> h^T = W1^T-natural matmuls (+b1),
    BN batch stats via free-dim reduction + a 4KB AllReduce across cores,
    fused scale/shift/ReLU on ACT, final 5-wide matmul, +b2, DMA out.
"""

import os
import numpy as np
import ml_dtypes

B, L, V, H = 1024, 200, 100000, 512
NCORES = 8
PCORE = B // NCORES  # 128 examples per core
BUCKET = 32768  # dma_gather int16 index range per bucket
NBUCKETS = (V + BUCKET - 1) // BUCKET  # 4
BN_EPS = 1e-5
CHUNK_TILES = 8  # token-tiles per dma_gather call (HW caps ~1024 idx/call)
SEG_PAD = 200.0  # segment id for padding slots (never matches 0..127)
EMB_DT_BF16 = True  # gather/pool in bf16 (halves HBM traffic)

_CACHE = {}


# ----------------------------------------------------------------- host prep
def _assign_cores(lengths):
    """Balanced assignment: 128 examples per core, ~equal total tokens."""
    order = np.argsort(-lengths, kind="stable")
    loads = [0] * NCORES
    counts = [0] * NCORES
    cores = [[] for _ in range(NCORES)]
    for ex in order:
        c = min(
            (c for c in range(NCORES) if counts[c] < PCORE),
            key=lambda c: loads[c],
        )
        cores[c].append(int(ex))
        loads[c] += int(lengths[ex])
        counts[c] += 1
    return cores  # cores[c] = list of 128 original example indices


def _prep(tokens, lengths):
    """Build per-core gather indices / segment ids / call plan."""
    cores = _assign_cores(lengths)

    # per (core, bucket): list of (idx16, seg)
    percb = [[[] for _ in range(NBUCKETS)] for _ in range(NCORES)]
    for c in range(NCORES):
        for slot, ex in enumerate(cores[c]):
            n = int(lengths[ex])
            toks = tokens[ex, :n]
            bs = toks >> 15
            rs = toks & 0x7FFF
            for b, r in zip(bs, rs):
                percb[c][b].append((int(r), slot))

    # shared padded bucket sizes (tiles), so all cores run the same program
    bsz = []
    for b in range(NBUCKETS):
        mx = max(len(percb[c][b]) for c in range(NCORES))
        bsz.append(-(-mx // 128) * 128)  # round up to full 128-slot tiles
    n_slots = sum(bsz)
    n_tiles = n_slots // 128

    # gather call plan: (bucket_base_row, tile_offset, n_tiles_call)
    calls = []
    t0 = 0
    for b in range(NBUCKETS):
        bt = bsz[b] // 128
        done = 0
        while done < bt:
            nt = min(CHUNK_TILES, bt - done)
            calls.append((b * BUCKET, t0 + done, nt))
            done += nt
        t0 += bt

    idx16 = np.zeros((NCORES, 128, n_slots // 16), dtype=np.int16)
    msk = np.zeros((NCORES, 128, n_tiles, 128), dtype=ml_dtypes.bfloat16)
    leninv = np.zeros((NCORES, 128, 1), dtype=np.float32)
    for c in range(NCORES):
        flat_idx = np.zeros(n_slots, dtype=np.int16)
        flat_seg = np.full(n_slots, SEG_PAD, dtype=np.float32)
        off = 0
        for b in range(NBUCKETS):
            lst = percb[c][b]
            if lst:
                arr = np.asarray(lst, dtype=np.int64)
                flat_idx[off : off + len(lst)] = arr[:, 0].astype(np.int16)
                flat_seg[off : off + len(lst)] = arr[:, 1]
            off += bsz[b]
        # wrap: slot k -> [k % 16, k // 16], replicated to 128 partitions
        w = flat_idx.reshape(n_slots // 16, 16).T  # [16, n_slots//16]
        idx16[c] = np.tile(w, (8, 1))
        # one-hot segment mask: [p, t, s] = (seg(slot t*128+p) == s)
        segs = flat_seg.reshape(n_tiles, 128).T  # [p, t]
        msk[c] = (segs[:, :, None] == np.arange(128)[None, None, :]).astype(
            ml_dtypes.bfloat16
        )
        leninv[c, :, 0] = 1.0 / lengths[np.asarray(cores[c])].astype(np.float32)

    return cores, calls, n_tiles, idx16, msk, leninv


# -------------------------------------------------------------- device build
def _build(n_tiles, calls, emb_dt_np):
    import concourse.bacc as bacc
    import concourse.tile as tile
    import concourse.mybir as mybir

    emb_dt = mybir.dt.bfloat16 if emb_dt_np == ml_dtypes.bfloat16 else mybir.dt.float32
    f32 = mybir.dt.float32

    nc = bacc.Bacc(
        "TRN2",
        target_bir_lowering=False,
        debug=False,
        enable_asserts=False,
        num_devices=NCORES,
        dynamic_dma_scratch_size=32768,
    )

    emb_d = nc.dram_tensor("emb", [V, H], emb_dt, kind="ExternalInput")
    idx_d = nc.dram_tensor(
        "idx16", [128, n_tiles * 8], mybir.dt.int16, kind="ExternalInput"
    )
    msk_d = nc.dram_tensor(
        "msk", [128, n_tiles * 128], emb_dt, kind="ExternalInput"
    )
    leninv_d = nc.dram_tensor("leninv", [128, 1], f32, kind="ExternalInput")
    w1_d = nc.dram_tensor("W1", [H, H], f32, kind="ExternalInput")
    b1_d = nc.dram_tensor("b1c", [128, 4], f32, kind="ExternalInput")
    gamma_d = nc.dram_tensor("gammac", [128, 4], f32, kind="ExternalInput")
    beta_d = nc.dram_tensor("betac", [128, 4], f32, kind="ExternalInput")
    w2_d = nc.dram_tensor("W2", [H, 5], f32, kind="ExternalInput")
    b2_d = nc.dram_tensor("b2rep", [128, 5], f32, kind="ExternalInput")
    ident_d = nc.dram_tensor("ident", [128, 128], f32, kind="ExternalInput")
    out_d = nc.dram_tensor("out", [128, 5], f32, kind="ExternalOutput")

    with tile.TileContext(nc) as tc:
        with (
            tc.tile_pool(name="const", bufs=1) as cpool,
            tc.tile_pool(name="gbuf", bufs=4) as gpool,
            tc.tile_pool(name="mbuf", bufs=4) as mpool,
            tc.tile_pool(name="work", bufs=1) as wpool,
            tc.tile_pool(name="ppool", bufs=1, space="PSUM") as ppool,
            tc.tile_pool(name="tpsum", bufs=2, space="PSUM") as tppool,
            tc.tile_pool(name="opsum", bufs=1, space="PSUM") as opool,
            tc.tile_pool(name="dram", bufs=1, space="DRAM") as dpool,
        ):
            # ---- constant / input loads
            idx_sb = cpool.tile([128, n_tiles * 8], mybir.dt.int16, tag="idx")
            leninv_sb = cpool.tile([128, 1], f32, tag="leninv")
            ident_sb = cpool.tile([128, 128], f32, tag="ident")
            b1_sb = cpool.tile([128, 4], f32, tag="b1")
            gamma_sb = cpool.tile([128, 4], f32, tag="gamma")
            beta_sb = cpool.tile([128, 4], f32, tag="beta")
            b2_sb = cpool.tile([128, 5], f32, tag="b2")
            w2_sb = cpool.tile([128, 4, 5], f32, tag="w2")
            w1_sb = [
                cpool.tile([128, H], f32, tag=f"w1_{k}", name=f"w1_{k}")
                for k in range(4)
            ]

            nc.sync.dma_start(idx_sb[:], idx_d[:, :])
            nc.sync.dma_start(leninv_sb[:], leninv_d[:, :])
            nc.sync.dma_start(ident_sb[:], ident_d[:, :])
            nc.sync.dma_start(b1_sb[:], b1_d[:, :])
            nc.sync.dma_start(gamma_sb[:], gamma_d[:, :])
            nc.sync.dma_start(beta_sb[:], beta_d[:, :])
            nc.sync.dma_start(b2_sb[:], b2_d[:, :])
            for k in range(4):
                nc.sync.dma_start(w1_sb[k][:], w1_d[k * 128 : (k + 1) * 128, :])
                nc.sync.dma_start(w2_sb[:, k, :], w2_d[k * 128 : (k + 1) * 128, :])

            # ---- gather + pooling matmuls (accumulate pooled[seg, H] in PSUM)
            pooled_ps = ppool.tile([128, H], f32, tag="pooled")
            emb_ap = emb_d.ap()
            tile_idx = 0
            n_calls = len(calls)
            for ci, (base, t0, nt) in enumerate(calls):
                rows = min(BUCKET, V - base)
                g_sb = gpool.tile([128, CHUNK_TILES, H], emb_dt, tag="g")
                nidx = nt * 128
                nc.gpsimd.dma_gather(
                    out_ap=g_sb[:, :nt, :],
                    in_ap=emb_ap[base : base + rows, :],
                    idxs_ap=idx_sb[:, t0 * 8 : (t0 + nt) * 8],
                    num_idxs=nidx,
                    num_idxs_reg=nidx,
                    elem_size=H,
                )
                m_sb = mpool.tile([128, CHUNK_TILES * 128], emb_dt, tag="m")
                nc.sync.dma_start(
                    m_sb[:, : nt * 128], msk_d[:, t0 * 128 : (t0 + nt) * 128]
                )
                for j in range(nt):
                    nc.tensor.matmul(
                        pooled_ps[:],
                        lhsT=m_sb[:, j * 128 : (j + 1) * 128],
                        rhs=g_sb[:, j, :],
                        start=(tile_idx == 0),
                        stop=(tile_idx == n_tiles - 1),
                    )
                    tile_idx += 1

            # ---- pooled mean (x 1/len), transpose to [H, B]
            pooled_sb = wpool.tile([128, H], f32, tag="pooled_sb")
            nc.vector.tensor_scalar_mul(pooled_sb[:], pooled_ps[:], leninv_sb[:, :1])

            pooledT_sb = wpool.tile([128, 4, 128], f32, tag="pooledT")
            for c in range(4):
                tp = tppool.tile([128, 128], f32, tag="tp")
                nc.tensor.transpose(
                    tp[:], pooled_sb[:, c * 128 : (c + 1) * 128], ident_sb[:]
                )
                nc.vector.tensor_copy(pooledT_sb[:, c, :], tp[:])

            # ---- h^T = W1^T @ pooled^T (+b1), stats
            hT_sb = wpool.tile([128, 4, 128], f32, tag="hT")
            stats_sb = wpool.tile([128, 8], f32, tag="stats")
            sq_sb = wpool.tile([128, 128], f32, tag="sq")
            for m in range(4):
                hps = tppool.tile([128, 128], f32, tag="hps")
                for k in range(4):
                    nc.tensor.matmul(
                        hps[:],
                        lhsT=w1_sb[k][:, m * 128 : (m + 1) * 128],
                        rhs=pooledT_sb[:, k, :],
                        start=(k == 0),
                        stop=(k == 3),
                    )
                nc.scalar.activation(
                    out=hT_sb[:, m, :],
                    in_=hps[:],
                    func=mybir.ActivationFunctionType.Identity,
                    bias=b1_sb[:, m : m + 1],
                    scale=1.0,
                )
                nc.vector.reduce_sum(
                    stats_sb[:, m : m + 1],
                    hT_sb[:, m, :],
                    axis=mybir.AxisListType.X,
                )
                nc.scalar.activation(
                    out=sq_sb[:],
                    in_=hT_sb[:, m, :],
                    func=mybir.ActivationFunctionType.Square,
                    accum_out=stats_sb[:, 4 + m : 5 + m],
                )

            # ---- AllReduce the BN partial sums (4KB)
            cc_in = dpool.tile([128, 8], f32, tag="cc_in")
            cc_out = dpool.tile([128, 8], f32, tag="cc_out")
            nc.sync.dma_start(cc_in[:], stats_sb[:])
            nc.gpsimd.collective_compute(
                "AllReduce",
                mybir.AluOpType.add,
                replica_groups=[list(range(NCORES))],
                ins=[cc_in.opt()],
                outs=[cc_out.opt()],
            )
            gstats_sb = wpool.tile([128, 8], f32, tag="gstats")
            nc.sync.dma_start(gstats_sb[:], cc_out[:])

            # ---- BN constants: scale = gamma/sqrt(var+eps), shift = beta - mu*scale
            mu = wpool.tile([128, 4], f32, tag="mu")
            var = wpool.tile([128, 4], f32, tag="var")
            std = wpool.tile([128, 4], f32, tag="std")
            inv = wpool.tile([128, 4], f32, tag="inv")
            scale = wpool.tile([128, 4], f32, tag="scale")
            shift = wpool.tile([128, 4], f32, tag="shift")
            nc.vector.tensor_scalar_mul(mu[:], gstats_sb[:, 0:4], 1.0 / B)
            nc.vector.tensor_scalar_mul(var[:], gstats_sb[:, 4:8], 1.0 / B)
            nc.vector.tensor_tensor(
                out=std[:], in0=mu[:], in1=mu[:], op=mybir.AluOpType.mult
            )
            nc.vector.tensor_tensor(
                out=var[:], in0=var[:], in1=std[:], op=mybir.AluOpType.subtract
            )
            eps_sb = wpool.tile([128, 1], f32, tag="eps")
            nc.vector.memset(eps_sb[:], BN_EPS)
            nc.scalar.activation(
                out=std[:],
                in_=var[:],
                func=mybir.ActivationFunctionType.Sqrt,
                bias=eps_sb[:, :1],
                scale=1.0,
            )
            nc.vector.reciprocal(inv[:], std[:])
            nc.vector.tensor_tensor(
                out=scale[:], in0=inv[:], in1=gamma_sb[:], op=mybir.AluOpType.mult
            )
            nc.vector.tensor_tensor(
                out=shift[:], in0=mu[:], in1=scale[:], op=mybir.AluOpType.mult
            )
            nc.vector.tensor_tensor(
                out=shift[:], in0=beta_sb[:], in1=shift[:], op=mybir.AluOpType.subtract
            )

            # ---- ReLU(h*scale + shift), final matmul, +b2, store
            hn_sb = wpool.tile([128, 4, 128], f32, tag="hn")
            for m in range(4):
                nc.scalar.activation(
                    out=hn_sb[:, m, :],
                    in_=hT_sb[:, m, :],
                    func=mybir.ActivationFunctionType.Relu,
                    bias=shift[:, m : m + 1],
                    scale=scale[:, m : m + 1],
                )
            out_ps = opool.tile([128, 5], f32, tag="out_ps")
            for m in range(4):
                nc.tensor.matmul(
                    out_ps[:],
                    lhsT=hn_sb[:, m, :],
                    rhs=w2_sb[:, m, :],
                    start=(m == 0),
                    stop=(m == 3),
                )
            out_sb = wpool.tile([128, 5], f32, tag="out_sb")
            nc.vector.tensor_tensor(
                out=out_sb[:], in0=out_ps[:], in1=b2_sb[:], op=mybir.AluOpType.add
            )
            nc.sync.dma_start(out_d[:, :], out_sb[:])

    nc.compile()
    return nc


# ------------------------------------------------------------------- runner
def _prepare(inputs):
    tokens = np.asarray(inputs["tokens"], dtype=np.int32)
    lengths = np.asarray(inputs["lengths"], dtype=np.int32)
    emb = np.asarray(inputs["emb"], dtype=np.float32)
    W1 = np.ascontiguousarray(np.asarray(inputs["W1"], dtype=np.float32))
    b1 = np.asarray(inputs["b1"], dtype=np.float32)
    gamma = np.asarray(inputs["gamma"], dtype=np.float32)
    beta = np.asarray(inputs["beta"], dtype=np.float32)
    W2 = np.ascontiguousarray(np.asarray(inputs["W2"], dtype=np.float32))
    b2 = np.asarray(inputs["b2"], dtype=np.float32)

    cores, calls, n_tiles, idx16, msk, leninv = _prep(tokens, lengths)

    emb_dt_np = ml_dtypes.bfloat16 if EMB_DT_BF16 else np.float32
    key = (n_tiles, tuple(calls), EMB_DT_BF16)
    if key not in _CACHE:
        _CACHE[key] = _build(n_tiles, calls, emb_dt_np)
    nc = _CACHE[key]

    emb_c = np.ascontiguousarray(emb.astype(emb_dt_np))
    ident = np.eye(128, dtype=np.float32)
    b1c = np.ascontiguousarray(b1.reshape(4, 128).T)
    gammac = np.ascontiguousarray(gamma.reshape(4, 128).T)
    betac = np.ascontiguousarray(beta.reshape(4, 128).T)
    b2rep = np.tile(b2.reshape(1, 5), (128, 1))

    in_maps = []
    for c in range(NCORES):
        in_maps.append(
            {
                "emb": emb_c,
                "idx16": np.ascontiguousarray(idx16[c]),
                "msk": np.ascontiguousarray(msk[c].reshape(128, n_tiles * 128)),
                "leninv": np.ascontiguousarray(leninv[c]),
                "W1": W1,
                "b1c": b1c,
                "gammac": gammac,
                "betac": betac,
                "W2": W2,
                "b2rep": b2rep,
                "ident": ident,
            }
        )
    return nc, in_maps, cores


def _run(inputs, trace=False):
    nc, in_maps, cores = _prepare(inputs)

    from concourse.bass_utils import run_bass_kernel_spmd

    res = run_bass_kernel_spmd(
        nc, in_maps, core_ids=list(range(NCORES)), trace=trace
    )

    out = np.zeros((B, 5), dtype=np.float32)
    for c in range(NCORES):
        out[np.asarray(cores[c])] = res.results[c]["out"]
    return out, res


def kernel(**inputs) -> np.ndarray:
    out, _ = _run(inputs, trace=False)
    return out


# revision 9
# speedup vs baseline: 1.3850x; 1.3543x over previous
"""Trainium2 Bass kernel for a bag-of-words model (EmbeddingBag mean ->
Linear -> BatchNorm(train stats) -> ReLU -> Linear).

Strategy (8 NeuronCores, SPMD):
  - Data-parallel over batch: 1024 examples -> 128 per core, assignment
    balanced by token count. Embedding table replicated per core (bf16).
  - Host compacts the ragged token lists (drops padding), buckets tokens by
    vocab range of 32768 rows so indices fit dma_gather's int16, and builds
    per-token-slot segment ids.
  - Device: dma_gather pulls only the valid embedding rows into SBUF
    (128 token-slots per partition-tile); pooling is done on TensorE as
    pooled[seg, H] += M_tile^T-free @ G_tile with M built on DVE via
    is_equal(iota, seg_id) (an exact 0/1 mask), accumulating in PSUM.
  - MLP: pooled/len -> PE transpose -> h^T = W1^T-natural matmuls (+b1),
    BN batch stats via free-dim reduction + a 4KB AllReduce across cores,
    fused scale/shift/ReLU on ACT, final 5-wide matmul, +b2, DMA out.
"""

import os
import numpy as np
import ml_dtypes

B, L, V, H = 1024, 200, 100000, 512
NCORES = 8
PCORE = B // NCORES  # 128 examples per core
BUCKET = 32768  # dma_gather int16 index range per bucket
NBUCKETS = (V + BUCKET - 1) // BUCKET  # 4
BN_EPS = 1e-5
CHUNK_TILES = 8  # token-tiles per dma_gather call (HW caps ~1024 idx/call)
SEG_PAD = 200.0  # segment id for padding slots (never matches 0..127)
EMB_DT_BF16 = True  # gather/pool in bf16 (halves HBM traffic)

_CACHE = {}


# ----------------------------------------------------------------- host prep
def _assign_cores(lengths):
    """Balanced assignment: 128 examples per core, ~equal total tokens."""
    order = np.argsort(-lengths, kind="stable")
    loads = [0] * NCORES
    counts = [0] * NCORES
    cores = [[] for _ in range(NCORES)]
    for ex in order:
        c = min(
            (c for c in range(NCORES) if counts[c] < PCORE),
            key=lambda c: loads[c],
        )
        cores[c].append(int(ex))
        loads[c] += int(lengths[ex])
        counts[c] += 1
    return cores  # cores[c] = list of 128 original example indices


def _prep(tokens, lengths):
    """Build per-core gather indices / segment ids / call plan."""
    cores = _assign_cores(lengths)

    # per (core, bucket): list of (idx16, seg)
    percb = [[[] for _ in range(NBUCKETS)] for _ in range(NCORES)]
    for c in range(NCORES):
        for slot, ex in enumerate(cores[c]):
            n = int(lengths[ex])
            toks = tokens[ex, :n]
            bs = toks >> 15
            rs = toks & 0x7FFF
            for b, r in zip(bs, rs):
                percb[c][b].append((int(r), slot))

    # shared padded bucket sizes (tiles), so all cores run the same program
    bsz = []
    for b in range(NBUCKETS):
        mx = max(len(percb[c][b]) for c in range(NCORES))
        bsz.append(-(-mx // 128) * 128)  # round up to full 128-slot tiles
    n_slots = sum(bsz)
    n_tiles = n_slots // 128

    # gather call plan: (bucket_base_row, tile_offset, n_tiles_call)
    calls = []
    t0 = 0
    for b in range(NBUCKETS):
        bt = bsz[b] // 128
        done = 0
        while done < bt:
            nt = min(CHUNK_TILES, bt - done)
            calls.append((b * BUCKET, t0 + done, nt))
            done += nt
        t0 += bt

    idx16 = np.zeros((NCORES, 128, n_slots // 16), dtype=np.int16)
    msk = np.zeros((NCORES, 128, n_tiles, 128), dtype=ml_dtypes.bfloat16)
    leninv = np.zeros((NCORES, 128, 1), dtype=np.float32)
    for c in range(NCORES):
        flat_idx = np.zeros(n_slots, dtype=np.int16)
        flat_seg = np.full(n_slots, SEG_PAD, dtype=np.float32)
        off = 0
        for b in range(NBUCKETS):
            lst = percb[c][b]
            if lst:
                arr = np.asarray(lst, dtype=np.int64)
                flat_idx[off : off + len(lst)] = arr[:, 0].astype(np.int16)
                flat_seg[off : off + len(lst)] = arr[:, 1]
            off += bsz[b]
        # wrap: slot k -> [k % 16, k // 16], replicated to 128 partitions
        w = flat_idx.reshape(n_slots // 16, 16).T  # [16, n_slots//16]
        idx16[c] = np.tile(w, (8, 1))
        # one-hot segment mask: [p, t, s] = (seg(slot t*128+p) == s)
        segs = flat_seg.reshape(n_tiles, 128).T  # [p, t]
        msk[c] = (segs[:, :, None] == np.arange(128)[None, None, :]).astype(
            ml_dtypes.bfloat16
        )
        leninv[c, :, 0] = 1.0 / lengths[np.asarray(cores[c])].astype(np.float32)

    return cores, calls, n_tiles, idx16, msk, leninv


# -------------------------------------------------------------- device build
def _build(n_tiles, calls, emb_dt_np):
    import concourse.bacc as bacc
    import concourse.tile as tile
    import concourse.mybir as mybir

    emb_dt = mybir.dt.bfloat16 if emb_dt_np == ml_dtypes.bfloat16 else mybir.dt.float32
    f32 = mybir.dt.float32

    nc = bacc.Bacc(
        "TRN2",
        target_bir_lowering=False,
        debug=False,
        enable_asserts=False,
        num_devices=NCORES,
        dynamic_dma_scratch_size=32768,
        num_swdge_queues=4,
    )

    emb_d = nc.dram_tensor("emb", [V, H], emb_dt, kind="ExternalInput")
    idx_d = nc.dram_tensor(
        "idx16", [128, n_tiles * 8], mybir.dt.int16, kind="ExternalInput"
    )
    msk_d = nc.dram_tensor(
        "msk", [128, n_tiles * 128], emb_dt, kind="ExternalInput"
    )
    leninv_d = nc.dram_tensor("leninv", [128, 1], f32, kind="ExternalInput")
    w1_d = nc.dram_tensor("W1", [H, H], emb_dt, kind="ExternalInput")
    b1_d = nc.dram_tensor("b1c", [128, 4], f32, kind="ExternalInput")
    gamma_d = nc.dram_tensor("gammac", [128, 4], f32, kind="ExternalInput")
    beta_d = nc.dram_tensor("betac", [128, 4], f32, kind="ExternalInput")
    w2_d = nc.dram_tensor("W2", [H, 5], f32, kind="ExternalInput")
    b2_d = nc.dram_tensor("b2rep", [128, 5], f32, kind="ExternalInput")
    ident_d = nc.dram_tensor("ident", [128, 128], emb_dt, kind="ExternalInput")
    out_d = nc.dram_tensor("out", [128, 5], f32, kind="ExternalOutput")

    with tile.TileContext(nc) as tc:
        with (
            tc.tile_pool(name="const", bufs=1) as cpool,
            tc.tile_pool(name="gbuf", bufs=4) as gpool,
            tc.tile_pool(name="mbuf", bufs=4) as mpool,
            tc.tile_pool(name="work", bufs=1) as wpool,
            tc.tile_pool(name="ppool", bufs=1, space="PSUM") as ppool,
            tc.tile_pool(name="tpsum", bufs=2, space="PSUM") as tppool,
            tc.tile_pool(name="opsum", bufs=1, space="PSUM") as opool,
            tc.tile_pool(name="dram", bufs=1, space="DRAM") as dpool,
        ):
            # ---- constant / input loads
            idx_sb = cpool.tile([128, n_tiles * 8], mybir.dt.int16, tag="idx")
            leninv_sb = cpool.tile([128, 1], f32, tag="leninv")
            ident_sb = cpool.tile([128, 128], emb_dt, tag="ident")
            b1_sb = cpool.tile([128, 4], f32, tag="b1")
            gamma_sb = cpool.tile([128, 4], f32, tag="gamma")
            beta_sb = cpool.tile([128, 4], f32, tag="beta")
            b2_sb = cpool.tile([128, 5], f32, tag="b2")
            w2_sb = cpool.tile([128, 4, 5], f32, tag="w2")
            w1_sb = [
                cpool.tile([128, H], emb_dt, tag=f"w1_{k}", name=f"w1_{k}")
                for k in range(4)
            ]

            nc.sync.dma_start(idx_sb[:], idx_d[:, :])
            nc.sync.dma_start(leninv_sb[:], leninv_d[:, :])
            nc.sync.dma_start(ident_sb[:], ident_d[:, :])
            nc.sync.dma_start(b1_sb[:], b1_d[:, :])
            nc.sync.dma_start(gamma_sb[:], gamma_d[:, :])
            nc.sync.dma_start(beta_sb[:], beta_d[:, :])
            nc.sync.dma_start(b2_sb[:], b2_d[:, :])
            for k in range(4):
                nc.sync.dma_start(w1_sb[k][:], w1_d[k * 128 : (k + 1) * 128, :])
                nc.sync.dma_start(w2_sb[:, k, :], w2_d[k * 128 : (k + 1) * 128, :])

            # ---- warm-up AllReduce: the first collective pays ~19us of ncfw
            # setup; fire a tiny dummy early so the real one is cheap.
            warm_in = dpool.tile([128, 1], f32, tag="warm_in")
            warm_out = dpool.tile([128, 1], f32, tag="warm_out")
            warm_sb = wpool.tile([128, 1], f32, tag="warm_sb")
            nc.vector.memset(warm_sb[:], 0.0)
            nc.sync.dma_start(warm_in[:], warm_sb[:])
            nc.gpsimd.collective_compute(
                "AllReduce",
                mybir.AluOpType.add,
                replica_groups=[list(range(NCORES))],
                ins=[warm_in.opt()],
                outs=[warm_out.opt()],
            )

            # ---- gather + pooling matmuls (accumulate pooled[seg, H] in PSUM)
            pooled_ps = ppool.tile([128, H], f32, tag="pooled")
            emb_ap = emb_d.ap()
            tile_idx = 0
            n_calls = len(calls)
            for ci, (base, t0, nt) in enumerate(calls):
                rows = min(BUCKET, V - base)
                g_sb = gpool.tile([128, CHUNK_TILES, H], emb_dt, tag="g")
                nidx = nt * 128
                nc.gpsimd.dma_gather(
                    out_ap=g_sb[:, :nt, :],
                    in_ap=emb_ap[base : base + rows, :],
                    idxs_ap=idx_sb[:, t0 * 8 : (t0 + nt) * 8],
                    num_idxs=nidx,
                    num_idxs_reg=nidx,
                    elem_size=H,
                    queue_num=ci % 4,
                )
                m_sb = mpool.tile([128, CHUNK_TILES * 128], emb_dt, tag="m")
                nc.sync.dma_start(
                    m_sb[:, : nt * 128], msk_d[:, t0 * 128 : (t0 + nt) * 128]
                )
                for j in range(nt):
                    nc.tensor.matmul(
                        pooled_ps[:],
                        lhsT=m_sb[:, j * 128 : (j + 1) * 128],
                        rhs=g_sb[:, j, :],
                        start=(tile_idx == 0),
                        stop=(tile_idx == n_tiles - 1),
                    )
                    tile_idx += 1

            # ---- pooled mean (x 1/len), transpose to [H, B]
            pooled_sb = wpool.tile([128, H], emb_dt, tag="pooled_sb")
            nc.vector.tensor_scalar_mul(pooled_sb[:], pooled_ps[:], leninv_sb[:, :1])

            pooledT_sb = wpool.tile([128, 4, 128], emb_dt, tag="pooledT")
            for c in range(4):
                tp = tppool.tile([128, 128], emb_dt, tag="tp")
                nc.tensor.transpose(
                    tp[:], pooled_sb[:, c * 128 : (c + 1) * 128], ident_sb[:]
                )
                nc.vector.tensor_copy(pooledT_sb[:, c, :], tp[:])

            # ---- h^T = W1^T @ pooled^T (+b1), stats
            hT_sb = wpool.tile([128, 4, 128], f32, tag="hT")
            stats_sb = wpool.tile([128, 8], f32, tag="stats")
            sq_sb = wpool.tile([128, 128], f32, tag="sq")
            for m in range(4):
                hps = tppool.tile([128, 128], f32, tag="hps")
                for k in range(4):
                    nc.tensor.matmul(
                        hps[:],
                        lhsT=w1_sb[k][:, m * 128 : (m + 1) * 128],
                        rhs=pooledT_sb[:, k, :],
                        start=(k == 0),
                        stop=(k == 3),
                    )
                nc.scalar.activation(
                    out=hT_sb[:, m, :],
                    in_=hps[:],
                    func=mybir.ActivationFunctionType.Identity,
                    bias=b1_sb[:, m : m + 1],
                    scale=1.0,
                )
                nc.vector.reduce_sum(
                    stats_sb[:, m : m + 1],
                    hT_sb[:, m, :],
                    axis=mybir.AxisListType.X,
                )
                nc.scalar.activation(
                    out=sq_sb[:],
                    in_=hT_sb[:, m, :],
                    func=mybir.ActivationFunctionType.Square,
                    accum_out=stats_sb[:, 4 + m : 5 + m],
                )

            # ---- AllReduce the BN partial sums (4KB)
            cc_in = dpool.tile([128, 8], f32, tag="cc_in")
            cc_out = dpool.tile([128, 8], f32, tag="cc_out")
            nc.sync.dma_start(cc_in[:], stats_sb[:])
            nc.gpsimd.collective_compute(
                "AllReduce",
                mybir.AluOpType.add,
                replica_groups=[list(range(NCORES))],
                ins=[cc_in.opt()],
                outs=[cc_out.opt()],
            )
            gstats_sb = wpool.tile([128, 8], f32, tag="gstats")
            nc.sync.dma_start(gstats_sb[:], cc_out[:])

            # ---- BN constants: scale = gamma/sqrt(var+eps), shift = beta - mu*scale
            mu = wpool.tile([128, 4], f32, tag="mu")
            var = wpool.tile([128, 4], f32, tag="var")
            std = wpool.tile([128, 4], f32, tag="std")
            inv = wpool.tile([128, 4], f32, tag="inv")
            scale = wpool.tile([128, 4], f32, tag="scale")
            shift = wpool.tile([128, 4], f32, tag="shift")
            nc.vector.tensor_scalar_mul(mu[:], gstats_sb[:, 0:4], 1.0 / B)
            nc.vector.tensor_scalar_mul(var[:], gstats_sb[:, 4:8], 1.0 / B)
            nc.vector.tensor_tensor(
                out=std[:], in0=mu[:], in1=mu[:], op=mybir.AluOpType.mult
            )
            nc.vector.tensor_tensor(
                out=var[:], in0=var[:], in1=std[:], op=mybir.AluOpType.subtract
            )
            eps_sb = wpool.tile([128, 1], f32, tag="eps")
            nc.vector.memset(eps_sb[:], BN_EPS)
            nc.scalar.activation(
                out=std[:],
                in_=var[:],
                func=mybir.ActivationFunctionType.Sqrt,
                bias=eps_sb[:, :1],
                scale=1.0,
            )
            nc.vector.reciprocal(inv[:], std[:])
            nc.vector.tensor_tensor(
                out=scale[:], in0=inv[:], in1=gamma_sb[:], op=mybir.AluOpType.mult
            )
            nc.vector.tensor_tensor(
                out=shift[:], in0=mu[:], in1=scale[:], op=mybir.AluOpType.mult
            )
            nc.vector.tensor_tensor(
                out=shift[:], in0=beta_sb[:], in1=shift[:], op=mybir.AluOpType.subtract
            )

            # ---- ReLU(h*scale + shift), final matmul, +b2, store
            hn_sb = wpool.tile([128, 4, 128], f32, tag="hn")
            for m in range(4):
                nc.scalar.activation(
                    out=hn_sb[:, m, :],
                    in_=hT_sb[:, m, :],
                    func=mybir.ActivationFunctionType.Relu,
                    bias=shift[:, m : m + 1],
                    scale=scale[:, m : m + 1],
                )
            out_ps = opool.tile([128, 5], f32, tag="out_ps")
            for m in range(4):
                nc.tensor.matmul(
                    out_ps[:],
                    lhsT=hn_sb[:, m, :],
                    rhs=w2_sb[:, m, :],
                    start=(m == 0),
                    stop=(m == 3),
                )
            out_sb = wpool.tile([128, 5], f32, tag="out_sb")
            nc.vector.tensor_tensor(
                out=out_sb[:], in0=out_ps[:], in1=b2_sb[:], op=mybir.AluOpType.add
            )
            nc.sync.dma_start(out_d[:, :], out_sb[:])

    nc.compile()
    return nc


# ------------------------------------------------------------------- runner
def _prepare(inputs):
    tokens = np.asarray(inputs["tokens"], dtype=np.int32)
    lengths = np.asarray(inputs["lengths"], dtype=np.int32)
    emb = np.asarray(inputs["emb"], dtype=np.float32)
    W1 = np.ascontiguousarray(np.asarray(inputs["W1"], dtype=np.float32))
    b1 = np.asarray(inputs["b1"], dtype=np.float32)
    gamma = np.asarray(inputs["gamma"], dtype=np.float32)
    beta = np.asarray(inputs["beta"], dtype=np.float32)
    W2 = np.ascontiguousarray(np.asarray(inputs["W2"], dtype=np.float32))
    b2 = np.asarray(inputs["b2"], dtype=np.float32)

    cores, calls, n_tiles, idx16, msk, leninv = _prep(tokens, lengths)

    emb_dt_np = ml_dtypes.bfloat16 if EMB_DT_BF16 else np.float32
    key = (n_tiles, tuple(calls), EMB_DT_BF16)
    if key not in _CACHE:
        _CACHE[key] = _build(n_tiles, calls, emb_dt_np)
    nc = _CACHE[key]

    emb_c = np.ascontiguousarray(emb.astype(emb_dt_np))
    ident = np.eye(128, dtype=np.float32).astype(emb_dt_np)
    b1c = np.ascontiguousarray(b1.reshape(4, 128).T)
    gammac = np.ascontiguousarray(gamma.reshape(4, 128).T)
    betac = np.ascontiguousarray(beta.reshape(4, 128).T)
    b2rep = np.tile(b2.reshape(1, 5), (128, 1))

    in_maps = []
    for c in range(NCORES):
        in_maps.append(
            {
                "emb": emb_c,
                "idx16": np.ascontiguousarray(idx16[c]),
                "msk": np.ascontiguousarray(msk[c].reshape(128, n_tiles * 128)),
                "leninv": np.ascontiguousarray(leninv[c]),
                "W1": W1.astype(emb_dt_np),
                "b1c": b1c,
                "gammac": gammac,
                "betac": betac,
                "W2": W2,
                "b2rep": b2rep,
                "ident": ident,
            }
        )
    return nc, in_maps, cores


def _run(inputs, trace=False):
    nc, in_maps, cores = _prepare(inputs)

    from concourse.bass_utils import run_bass_kernel_spmd

    res = run_bass_kernel_spmd(
        nc, in_maps, core_ids=list(range(NCORES)), trace=trace
    )

    out = np.zeros((B, 5), dtype=np.float32)
    for c in range(NCORES):
        out[np.asarray(cores[c])] = res.results[c]["out"]
    return out, res


def kernel(**inputs) -> np.ndarray:
    out, _ = _run(inputs, trace=False)
    return out
